# revision 9
# baseline (speedup 1.0000x reference)
"""BlockAttentionResidual Trainium2 kernel.

Math (per (b,t) row, V slice v_n of length D, n = 0..7):
    ssq_n = sum(v_n^2)
    rms_n = rsqrt(ssq_n / D + eps)
    logit_n = rms_n * dot(v_n, qw)        with qw = key_norm_weight * pseudo_query
    w = softmax(logit)                     over n
    out = sum_n w_n * v_n

Sharding: rows (B*T flattened) split evenly across 8 cores; (D,) params
replicated. No cross-device communication.

Key layout/precision choices (measured on trn2):
  - V is cast to fp16 on the host and pre-interleaved into the exact SBUF
    tile layout (tile t, partition 8r+n = rows 2r,2r+1 of plane n), so
    device input DMAs are fully sequential HBM reads.  The interleaved
    (plane-strided) read pattern costs ~40% DMA bandwidth; sequential
    reads run at full speed.  fp16 halves HBM traffic; whole-pipeline
    fp16 rel err vs the f32 reference is ~1.2e-3 (gate 2e-2).
  - ssq: ScalarE activation(Square, accum_out); dot: VectorE
    scalar_tensor_tensor(mult, accum_out), per 2048-elem row-half.
  - rms = exp(-0.5*ln(ssq/D+eps)) on ScalarE.
  - softmax over n: PE-transpose the (128, 2G) scalar columns so n lands
    innermost on the free axis, then max/exp/sum/div, transpose back.
  - weighted sum: PE matmul in fp16; banded (128, 128) stationaries
    place row 32c+2r+eo at PSUM partition 32c+2r+eo, accumulating
    matmuls per 512-chunk, groups of 4 tiles (last 4 tiles as
    single-tile groups to shrink the pipeline tail); PSUM->SBUF copy
    split ACT/DVE; fp16 output stores, host converts back to f32.
DMA rings: input tiles on the SP HWDGE ring, consts + output stores on
the ACT HWDGE ring.
"""

import os
import sys

for _p in ("/opt/trn_rl_repo",):
    if _p not in sys.path and os.path.isdir(_p):
        sys.path.append(_p)

import ml_dtypes
import numpy as np

import concourse.bass as bass
import concourse.tile as tile
from concourse import bacc, mybir
from concourse.bass_utils import run_bass_kernel_spmd

N_CORES = 8
N = 8          # depth entries (softmax axis)
B = 4
T = 2048
D = 2048
R_TOTAL = B * T            # 8192 rows
RPC = R_TOTAL // N_CORES   # 1024 rows per core
TR = 32                    # rows per tile (16 r-slots x 2 rows x 8 n)
EPS = 1e-6
NCHUNK = 512               # matmul moving free-dim chunk (PSUM bank width)

F32 = mybir.dt.float32
BF16 = mybir.dt.bfloat16
ALU = mybir.AluOpType
ACTF = mybir.ActivationFunctionType


def _patch_act_tables():
    """Make every ACT function resolve to `natural_log_exp_and_others`.

    The table-load pass picks the first act-func set containing each
    function, which alternates exp/square (set 0) and ln (set 5) and
    inserts a ~1.3us ACT_TABLE_LOAD per switch.  Set 6 genuinely contains
    exp+ln+square+copy, so steering everything there leaves one load
    total.  Only the selection input is patched; walrus still emits the
    real act_info tables.
    """
    if getattr(bacc, "_act_tables_patched", False):
        return
    orig = bacc.get_activation_tables

    def patched(arch):
        tabs = orig(arch)
        target = "natural_log_exp_and_others"
        if target in tabs:
            exp = mybir.ActivationFunctionType.Exp
            ln = mybir.ActivationFunctionType.Ln
            for name, funcs in tabs.items():
                if name != target:
                    funcs.discard(exp)
                    funcs.discard(ln)
        return tabs

    bacc.get_activation_tables = patched
    bacc._act_tables_patched = True


def build_program(rows_per_core=RPC, debug=False, xbufs=9):
    """Build the per-core Bass program (identical on all cores)."""
    _patch_act_tables()
    nt = rows_per_core // TR           # tiles per core
    nc = bacc.Bacc(
        "TRN2", target_bir_lowering=False, debug=debug, num_devices=N_CORES
    )

    v_dram = nc.dram_tensor(
        "V", (nt, 128, 2 * D), BF16, kind="ExternalInput"
    ).ap()
    qw_dram = nc.dram_tensor("QW", (128, D), BF16, kind="ExternalInput").ap()
    em_dram = nc.dram_tensor("EM", (8, 128, 128), BF16, kind="ExternalInput").ap()
    id_dram = nc.dram_tensor("ID", (128, 128), F32, kind="ExternalInput").ap()
    out_dram = nc.dram_tensor(
        "OUT", (rows_per_core, D), BF16, kind="ExternalOutput"
    ).ap()

    with tile.TileContext(nc) as tc:
        with (
            tc.tile_pool(name="consts", bufs=1) as consts,
            tc.tile_pool(name="xpool", bufs=xbufs) as xpool,
            tc.tile_pool(name="scratch", bufs=1) as scratch,
            tc.tile_pool(name="outpool", bufs=2) as outpool,
            tc.tile_pool(name="smalls", bufs=3) as smalls,
            tc.tile_pool(name="wdpool", bufs=16) as wdpool,
            tc.tile_pool(name="psum_big", bufs=2, space="PSUM") as psum_big_pool,
            tc.tile_pool(name="psum_sm", bufs=2, space="PSUM") as psum_sm,
        ):
            qw_sb = consts.tile([128, D], BF16)
            nc.scalar.dma_start(qw_sb[:], qw_dram[:])
            em_sb = []
            for i in range(8):
                em = consts.tile([128, 128], BF16, tag=f"em{i}")
                nc.scalar.dma_start(em[:], em_dram[i])
                em_sb.append(em)
            id_sb = consts.tile([128, 128], F32)
            nc.scalar.dma_start(id_sb[:], id_dram[:])
            zero_sb = consts.tile([128, 1], F32)
            nc.vector.memset(zero_sb[:], 0.0)
            eps_sb = consts.tile([128, 1], F32)
            nc.vector.memset(eps_sb[:], EPS)

            assert nt % 4 == 0, "tiles per core must be a multiple of 4"

            def emit_group(tlist):
                G = len(tlist)
                xt = []
                dots = smalls.tile([128, 2 * G], F32, tag="dots")
                ssqs = smalls.tile([128, 2 * G], F32, tag="ssqs")
                for j, t in enumerate(tlist):
                    x = xpool.tile([128, 2 * D], BF16, tag="x")
                    # host pre-interleaved: v_dram[t][8r+n] = rows (2r, 2r+1)
                    # of plane n -> fully sequential HBM reads
                    nc.sync.dma_start(x[:], v_dram[t])
                    xt.append(x)

                    for eo in range(2):
                        xh = x[:, D * eo : D * (eo + 1)]
                        prod = scratch.tile([128, D], BF16, tag="prod")
                        nc.vector.scalar_tensor_tensor(
                            out=prod[:],
                            in0=xh,
                            scalar=1.0,
                            in1=qw_sb[:],
                            op0=ALU.mult,
                            op1=ALU.mult,
                            accum_out=dots[:, 2 * j + eo : 2 * j + eo + 1],
                        )
                        sq = scratch.tile([128, D], BF16, tag="sq")
                        nc.scalar.activation(
                            sq[:], xh, ACTF.Square, bias=zero_sb[:],
                            accum_out=ssqs[:, 2 * j + eo : 2 * j + eo + 1],
                        )

                # logits = dot * rsqrt(ssq/D + eps)  — (128, 2G)
                lns = smalls.tile([128, 2 * G], F32, tag="lns")
                nc.scalar.activation(
                    lns[:], ssqs[:], ACTF.Ln, bias=eps_sb[:], scale=1.0 / D
                )
                rms = smalls.tile([128, 2 * G], F32, tag="rms")
                nc.scalar.activation(
                    rms[:], lns[:], ACTF.Exp, bias=zero_sb[:], scale=-0.5
                )
                logits = smalls.tile([128, 2 * G], F32, tag="logits")
                nc.vector.tensor_mul(logits[:], dots[:], rms[:])

                # transpose to (2G, 128) so n is innermost on the free axis
                ps_t = psum_sm.tile([2 * G, 128], F32, tag="pst")
                nc.tensor.transpose(ps_t[:], logits[:], id_sb[:])
                tsb = smalls.tile([2 * G, 128], F32, tag="tsb")
                nc.scalar.copy(tsb[:], ps_t[:])
                t3 = tsb[:].rearrange("p (r n) -> p r n", n=N)

                negmax = smalls.tile([2 * G, 16], F32, tag="negmax")
                nc.vector.tensor_reduce(
                    negmax[:], t3, axis=mybir.AxisListType.X, op=ALU.max, negate=True
                )
                shifted = smalls.tile([2 * G, 128], F32, tag="shifted")
                sh3 = shifted[:].rearrange("p (r n) -> p r n", n=N)
                nmb = negmax[:].unsqueeze(2).broadcast_to([2 * G, 16, N])
                nc.vector.tensor_tensor(sh3, t3, nmb, ALU.add)
                expd = smalls.tile([2 * G, 128], F32, tag="expd")
                nc.scalar.activation(expd[:], shifted[:], ACTF.Exp, bias=zero_sb[0 : 2 * G])
                ex3 = expd[:].rearrange("p (r n) -> p r n", n=N)
                sums = smalls.tile([2 * G, 16], F32, tag="sums")
                nc.vector.tensor_reduce(
                    sums[:], ex3, axis=mybir.AxisListType.X, op=ALU.add
                )
                rsums = smalls.tile([2 * G, 16], F32, tag="rsums")
                nc.vector.reciprocal(rsums[:], sums[:])
                wts = smalls.tile([2 * G, 128], F32, tag="wts")
                w3 = wts[:].rearrange("p (r n) -> p r n", n=N)
                rsb = rsums[:].unsqueeze(2).broadcast_to([2 * G, 16, N])
                nc.vector.tensor_tensor(w3, ex3, rsb, ALU.mult)

                # transpose back: column 2j+eo = weights for (tile j, parity eo)
                ps_w = psum_sm.tile([128, 2 * G], F32, tag="psw")
                nc.tensor.transpose(ps_w[:], wts[:], id_sb[0 : 2 * G, 0 : 2 * G])
                wcols = smalls.tile([128, 2 * G], F32, tag="wcols")
                nc.scalar.copy(wcols[:], ps_w[:])

                # weighted sum via PE: banded (128,128) fp16 stationaries,
                # 8 accumulating matmuls per 512-chunk (4 tiles x 2 parities)
                wds = []
                for c in range(G):
                    for eo in range(2):
                        wd = wdpool.tile([128, 128], BF16, tag="wd")
                        nc.gpsimd.tensor_scalar(
                            out=wd[:], in0=em_sb[2 * c + eo][:],
                            scalar1=wcols[:, 2 * c + eo : 2 * c + eo + 1],
                            scalar2=None, op0=ALU.mult,
                        )
                        wds.append(wd)
                osb = outpool.tile([32 * G, D], BF16, tag="osb")
                for h in range(2):
                    psb = psum_big_pool.tile([32 * G, D // 2], F32, tag="psb")
                    for kk in range(D // NCHUNK // 2):
                        k = h * (D // NCHUNK // 2) + kk
                        ps_slice = psb[:, NCHUNK * kk : NCHUNK * (kk + 1)]
                        for c in range(G):
                            for eo in range(2):
                                nc.tensor.matmul(
                                    ps_slice, wds[2 * c + eo][:, 0 : 32 * G],
                                    xt[c][:, D * eo + NCHUNK * k
                                           : D * eo + NCHUNK * (k + 1)],
                                    start=(c == 0 and eo == 0),
                                    stop=(c == G - 1 and eo == 1),
                                )
                    # split the PSUM->SBUF copy across ACT and DVE
                    eng = nc.scalar.copy if h == 0 else nc.vector.tensor_copy
                    eng(osb[:, h * (D // 2) : (h + 1) * (D // 2)], psb[:])
                # masks put row 32c+2r+eo at partition 32c+2r+eo: plain store
                r0 = TR * tlist[0]
                nc.scalar.dma_start(out_dram[r0 : r0 + 32 * G, :], osb[:])

            for g in range(nt // 4 - 1):
                emit_group([4 * g + j for j in range(4)])
            for t in range(nt - 4, nt):
                emit_group([t])

    nc.compile()
    return nc


def make_consts():
    """Host-side constants: even/odd block-diagonal masks and identity."""
    em = np.zeros((8, 128, 128), dtype=ml_dtypes.bfloat16)
    for c in range(4):
        for eo in range(2):
            for p in range(128):
                r = p // N
                em[2 * c + eo, p, 32 * c + 2 * r + eo] = 1.0
    ident = np.eye(128, dtype=np.float32)
    return em, ident


def prepare_in_maps(V, key_norm_weight, pseudo_query, rows_per_core=RPC,
                    n_cores=N_CORES):
    qw = (np.asarray(key_norm_weight, dtype=np.float32)
          * np.asarray(pseudo_query, dtype=np.float32)).astype(ml_dtypes.bfloat16)
    qw_b = np.ascontiguousarray(np.broadcast_to(qw, (128, D)))
    em, ident = make_consts()
    nt = rows_per_core // TR
    # pre-interleave to the SBUF tile layout: per core, tile t, partition
    # 8r+n holds rows (2r, 2r+1) of plane n -> device reads are sequential
    vf = np.asarray(V, dtype=np.float32).astype(ml_dtypes.bfloat16).reshape(
        N, n_cores, nt, 16, 2, D
    )
    vt = np.ascontiguousarray(vf.transpose(1, 2, 3, 0, 4, 5)).reshape(
        n_cores, nt, 128, 2 * D
    )
    in_maps = []
    for c in range(n_cores):
        in_maps.append({"V": vt[c], "QW": qw_b, "EM": em, "ID": ident})
    return in_maps


_PROGRAM_CACHE = {}


def _get_program():
    key = (RPC,)
    if key not in _PROGRAM_CACHE:
        _PROGRAM_CACHE[key] = build_program(RPC, debug=False)
    return _PROGRAM_CACHE[key]


def run(V, key_norm_weight, pseudo_query, trace=False, **trace_kwargs):
    nc = _get_program()
    in_maps = prepare_in_maps(V, key_norm_weight, pseudo_query)
    res = run_bass_kernel_spmd(
        nc, in_maps, list(range(N_CORES)), trace=trace, **trace_kwargs
    )
    out = np.empty((R_TOTAL, D), dtype=np.float32)
    for c in range(N_CORES):
        out[c * RPC : (c + 1) * RPC, :] = res.results[c]["OUT"].astype(np.float32)
    return out.reshape(B, T, D), res


def kernel(V, key_norm_weight, pseudo_query):
    out, _ = run(V, key_norm_weight, pseudo_query, trace=False)
    return out


# revision 15
# speedup vs baseline: 1.5161x; 1.5161x over previous
"""BlockAttentionResidual Trainium2 kernel.

Math (per (b,t) row, V slice v_n of length D, n = 0..7):
    ssq_n = sum(v_n^2)
    rms_n = rsqrt(ssq_n / D + eps)
    logit_n = rms_n * dot(v_n, qw)        with qw = key_norm_weight * pseudo_query
    w = softmax(logit)                     over n
    out = sum_n w_n * v_n

Sharding: rows (B*T flattened) split evenly across 8 cores; (D,) params
replicated. No cross-device communication.

Key layout/precision choices (measured on trn2):
  - V is cast to fp16 on the host and pre-interleaved into the exact SBUF
    tile layout (tile t, partition 8r+n = rows 2r,2r+1 of plane n), so
    device input DMAs are fully sequential HBM reads.  The interleaved
    (plane-strided) read pattern costs ~40% DMA bandwidth; sequential
    reads run at full speed.  fp16 halves HBM traffic; whole-pipeline
    fp16 rel err vs the f32 reference is ~1.2e-3 (gate 2e-2).
  - ssq: ScalarE activation(Square, accum_out); dot: VectorE
    scalar_tensor_tensor(mult, accum_out), per 2048-elem row-half.
  - rms = exp(-0.5*ln(ssq/D+eps)) on ScalarE.
  - softmax over n: PE-transpose the (128, 2G) scalar columns so n lands
    innermost on the free axis, then max/exp/sum/div, transpose back.
  - weighted sum: PE matmul in fp16; banded (128, 128) stationaries
    place row 32c+2r+eo at PSUM partition 32c+2r+eo, accumulating
    matmuls per 512-chunk, groups of 4 tiles (last 4 tiles as
    single-tile groups to shrink the pipeline tail); PSUM->SBUF copy
    split ACT/DVE; fp16 output stores, host converts back to f32.
DMA rings: input tiles on the SP HWDGE ring, consts + output stores on
the ACT HWDGE ring.
"""

import os
import sys

for _p in ("/opt/trn_rl_repo",):
    if _p not in sys.path and os.path.isdir(_p):
        sys.path.append(_p)

import ml_dtypes
import numpy as np

import concourse.bass as bass
import concourse.tile as tile
from concourse import bacc, mybir
from concourse.bass_utils import run_bass_kernel_spmd

N_CORES = 8
N = 8          # depth entries (softmax axis)
B = 4
T = 2048
D = 2048
R_TOTAL = B * T            # 8192 rows
RPC = R_TOTAL // N_CORES   # 1024 rows per core
TR = 32                    # rows per tile (16 r-slots x 2 rows x 8 n)
EPS = 1e-6
NCHUNK = 512               # matmul moving free-dim chunk (PSUM bank width)

F32 = mybir.dt.float32
BF16 = mybir.dt.bfloat16
ALU = mybir.AluOpType
ACTF = mybir.ActivationFunctionType


def _patch_act_tables():
    """Make every ACT function resolve to `natural_log_exp_and_others`.

    The table-load pass picks the first act-func set containing each
    function, which alternates exp/square (set 0) and ln (set 5) and
    inserts a ~1.3us ACT_TABLE_LOAD per switch.  Set 6 genuinely contains
    exp+ln+square+copy, so steering everything there leaves one load
    total.  Only the selection input is patched; walrus still emits the
    real act_info tables.
    """
    if getattr(bacc, "_act_tables_patched", False):
        return
    orig = bacc.get_activation_tables

    def patched(arch):
        tabs = orig(arch)
        target = "natural_log_exp_and_others"
        if target in tabs:
            exp = mybir.ActivationFunctionType.Exp
            ln = mybir.ActivationFunctionType.Ln
            for name, funcs in tabs.items():
                if name != target:
                    funcs.discard(exp)
                    funcs.discard(ln)
        return tabs

    bacc.get_activation_tables = patched
    bacc._act_tables_patched = True


def build_program(rows_per_core=RPC, debug=False, xbufs=9):
    """Build the per-core Bass program (identical on all cores)."""
    _patch_act_tables()
    nt = rows_per_core // TR           # tiles per core
    nc = bacc.Bacc(
        "TRN2", target_bir_lowering=False, debug=debug, num_devices=N_CORES
    )

    v_dram = nc.dram_tensor(
        "V", (nt, 128, 2 * D), BF16, kind="ExternalInput"
    ).ap()
    qw_dram = nc.dram_tensor("QW", (128, D), BF16, kind="ExternalInput").ap()
    em_dram = nc.dram_tensor("EM", (128, 8 * 128), BF16, kind="ExternalInput").ap()
    id_dram = nc.dram_tensor("ID", (128, 128), F32, kind="ExternalInput").ap()
    out_dram = nc.dram_tensor(
        "OUT", (rows_per_core, D), BF16, kind="ExternalOutput"
    ).ap()

    with tile.TileContext(nc) as tc:
        with (
            tc.tile_pool(name="consts", bufs=1) as consts,
            tc.tile_pool(name="xpool", bufs=xbufs) as xpool,
            tc.tile_pool(name="scratch", bufs=1) as scratch,
            tc.tile_pool(name="outpool", bufs=2) as outpool,
            tc.tile_pool(name="smalls", bufs=3) as smalls,
            tc.tile_pool(name="wdpool", bufs=16) as wdpool,
            tc.tile_pool(name="psum_big", bufs=2, space="PSUM") as psum_big_pool,
            tc.tile_pool(name="psum_sm", bufs=2, space="PSUM") as psum_sm,
        ):
            qw_sb = consts.tile([128, D], BF16)
            nc.scalar.dma_start(qw_sb[:], qw_dram[:])
            em_all = consts.tile([128, 8 * 128], BF16, tag="em_all")
            nc.scalar.dma_start(em_all[:], em_dram[:])
            id_sb = consts.tile([128, 128], F32)
            nc.scalar.dma_start(id_sb[:], id_dram[:])
            zero_sb = consts.tile([128, 1], F32)
            nc.vector.memset(zero_sb[:], 0.0)
            eps_sb = consts.tile([128, 1], F32)
            nc.vector.memset(eps_sb[:], EPS)

            assert nt % 4 == 0, "tiles per core must be a multiple of 4"

            def emit_group(tlist):
                G = len(tlist)
                xt = []
                dots = smalls.tile([128, 2 * G], F32, tag="dots")
                ssqs = smalls.tile([128, 2 * G], F32, tag="ssqs")
                for j, t in enumerate(tlist):
                    x = xpool.tile([128, 2 * D], BF16, tag="x")
                    # host pre-interleaved: v_dram[t][8r+n] = rows (2r, 2r+1)
                    # of plane n -> fully sequential HBM reads
                    nc.sync.dma_start(x[:], v_dram[t])
                    xt.append(x)

                    for eo in range(2):
                        xh = x[:, D * eo : D * (eo + 1)]
                        prod = scratch.tile([128, D], BF16, tag="prod")
                        nc.vector.scalar_tensor_tensor(
                            out=prod[:],
                            in0=xh,
                            scalar=1.0,
                            in1=qw_sb[:],
                            op0=ALU.mult,
                            op1=ALU.mult,
                            accum_out=dots[:, 2 * j + eo : 2 * j + eo + 1],
                        )
                        sq = scratch.tile([128, D], BF16, tag="sq")
                        nc.scalar.activation(
                            sq[:], xh, ACTF.Square, bias=zero_sb[:],
                            accum_out=ssqs[:, 2 * j + eo : 2 * j + eo + 1],
                        )

                # logits = dot * rsqrt(ssq/D + eps)  — (128, 2G)
                lns = smalls.tile([128, 2 * G], F32, tag="lns")
                nc.scalar.activation(
                    lns[:], ssqs[:], ACTF.Ln, bias=eps_sb[:], scale=1.0 / D
                )
                rms = smalls.tile([128, 2 * G], F32, tag="rms")
                nc.scalar.activation(
                    rms[:], lns[:], ACTF.Exp, bias=zero_sb[:], scale=-0.5
                )
                logits = smalls.tile([128, 2 * G], F32, tag="logits")
                nc.vector.tensor_mul(logits[:], dots[:], rms[:])

                # transpose to (2G, 128) so n is innermost on the free axis
                ps_t = psum_sm.tile([2 * G, 128], F32, tag="pst")
                nc.tensor.transpose(ps_t[:], logits[:], id_sb[:])
                tsb = smalls.tile([2 * G, 128], F32, tag="tsb")
                nc.scalar.copy(tsb[:], ps_t[:])
                t3 = tsb[:].rearrange("p (r n) -> p r n", n=N)

                negmax = smalls.tile([2 * G, 16], F32, tag="negmax")
                nc.vector.tensor_reduce(
                    negmax[:], t3, axis=mybir.AxisListType.X, op=ALU.max, negate=True
                )
                shifted = smalls.tile([2 * G, 128], F32, tag="shifted")
                sh3 = shifted[:].rearrange("p (r n) -> p r n", n=N)
                nmb = negmax[:].unsqueeze(2).broadcast_to([2 * G, 16, N])
                nc.vector.tensor_tensor(sh3, t3, nmb, ALU.add)
                expd = smalls.tile([2 * G, 128], F32, tag="expd")
                nc.scalar.activation(expd[:], shifted[:], ACTF.Exp, bias=zero_sb[0 : 2 * G])
                ex3 = expd[:].rearrange("p (r n) -> p r n", n=N)
                sums = smalls.tile([2 * G, 16], F32, tag="sums")
                nc.vector.tensor_reduce(
                    sums[:], ex3, axis=mybir.AxisListType.X, op=ALU.add
                )
                rsums = smalls.tile([2 * G, 16], F32, tag="rsums")
                nc.vector.reciprocal(rsums[:], sums[:])
                wts = smalls.tile([2 * G, 128], F32, tag="wts")
                w3 = wts[:].rearrange("p (r n) -> p r n", n=N)
                rsb = rsums[:].unsqueeze(2).broadcast_to([2 * G, 16, N])
                nc.vector.tensor_tensor(w3, ex3, rsb, ALU.mult)

                # transpose back: column 2j+eo = weights for (tile j, parity eo)
                ps_w = psum_sm.tile([128, 2 * G], F32, tag="psw")
                nc.tensor.transpose(ps_w[:], wts[:], id_sb[0 : 2 * G, 0 : 2 * G])
                wcols = smalls.tile([128, 2 * G], F32, tag="wcols")
                nc.scalar.copy(wcols[:], ps_w[:])

                # weighted sum via PE: banded (128,128) bf16 stationaries,
                # 8 accumulating matmuls per 512-chunk (4 tiles x 2 parities).
                # All 2G stationaries are built in one fused multiply on the
                # otherwise-idle GpSimd engine: wd_all[:, 128k + q] =
                # em[k][:, q] * wcols[:, k].
                wd_all = wdpool.tile([128, 2 * G * 128], BF16, tag="wd")
                wcb = (
                    wcols[:]
                    .unsqueeze(2)
                    .broadcast_to([128, 2 * G, 128])
                )
                nc.gpsimd.tensor_tensor(
                    wd_all[:].rearrange("p (k q) -> p k q", q=128),
                    em_all[:, 0 : 2 * G * 128].rearrange(
                        "p (k q) -> p k q", q=128
                    ),
                    wcb,
                    ALU.mult,
                )
                osb = outpool.tile([32 * G, D], BF16, tag="osb")
                for h in range(2):
                    psb = psum_big_pool.tile([32 * G, D // 2], F32, tag="psb")
                    for kk in range(D // NCHUNK // 2):
                        k = h * (D // NCHUNK // 2) + kk
                        ps_slice = psb[:, NCHUNK * kk : NCHUNK * (kk + 1)]
                        for c in range(G):
                            for eo in range(2):
                                nc.tensor.matmul(
                                    ps_slice,
                                    wd_all[:, 128 * (2 * c + eo)
                                           : 128 * (2 * c + eo) + 32 * G],
                                    xt[c][:, D * eo + NCHUNK * k
                                           : D * eo + NCHUNK * (k + 1)],
                                    start=(c == 0 and eo == 0),
                                    stop=(c == G - 1 and eo == 1),
                                )
                    # split the PSUM->SBUF copy across ACT and DVE
                    eng = nc.scalar.copy if h == 0 else nc.vector.tensor_copy
                    eng(osb[:, h * (D // 2) : (h + 1) * (D // 2)], psb[:])
                # masks put row 32c+2r+eo at partition 32c+2r+eo: plain store
                r0 = TR * tlist[0]
                nc.scalar.dma_start(out_dram[r0 : r0 + 32 * G, :], osb[:])

            for g in range(nt // 4 - 1):
                emit_group([4 * g + j for j in range(4)])
            for t in range(nt - 4, nt):
                emit_group([t])

    nc.compile()
    return nc


def make_consts():
    """Host-side constants: even/odd block-diagonal masks and identity.

    em[p, 128*(2c+eo) + q] = 1 iff q == 32c + 2*(p//8) + eo  (concatenated
    along the free axis so one fused multiply builds all stationaries).
    """
    em = np.zeros((128, 8 * 128), dtype=ml_dtypes.bfloat16)
    for c in range(4):
        for eo in range(2):
            for p in range(128):
                r = p // N
                em[p, 128 * (2 * c + eo) + 32 * c + 2 * r + eo] = 1.0
    ident = np.eye(128, dtype=np.float32)
    return em, ident


def prepare_in_maps(V, key_norm_weight, pseudo_query, rows_per_core=RPC,
                    n_cores=N_CORES):
    qw = (np.asarray(key_norm_weight, dtype=np.float32)
          * np.asarray(pseudo_query, dtype=np.float32)).astype(ml_dtypes.bfloat16)
    qw_b = np.ascontiguousarray(np.broadcast_to(qw, (128, D)))
    em, ident = make_consts()
    nt = rows_per_core // TR
    # pre-interleave to the SBUF tile layout: per core, tile t, partition
    # 8r+n holds rows (2r, 2r+1) of plane n -> device reads are sequential
    vf = np.asarray(V, dtype=np.float32).astype(ml_dtypes.bfloat16).reshape(
        N, n_cores, nt, 16, 2, D
    )
    vt = np.ascontiguousarray(vf.transpose(1, 2, 3, 0, 4, 5)).reshape(
        n_cores, nt, 128, 2 * D
    )
    in_maps = []
    for c in range(n_cores):
        in_maps.append({"V": vt[c], "QW": qw_b, "EM": em, "ID": ident})
    return in_maps


_PROGRAM_CACHE = {}


def _get_program():
    key = (RPC,)
    if key not in _PROGRAM_CACHE:
        _PROGRAM_CACHE[key] = build_program(RPC, debug=False)
    return _PROGRAM_CACHE[key]


def run(V, key_norm_weight, pseudo_query, trace=False, **trace_kwargs):
    nc = _get_program()
    in_maps = prepare_in_maps(V, key_norm_weight, pseudo_query)
    res = run_bass_kernel_spmd(
        nc, in_maps, list(range(N_CORES)), trace=trace, **trace_kwargs
    )
    out = np.empty((R_TOTAL, D), dtype=np.float32)
    for c in range(N_CORES):
        out[c * RPC : (c + 1) * RPC, :] = res.results[c]["OUT"].astype(np.float32)
    return out.reshape(B, T, D), res


def kernel(V, key_norm_weight, pseudo_query):
    out, _ = run(V, key_norm_weight, pseudo_query, trace=False)
    return out
